# revision 4
# baseline (speedup 1.0000x reference)
"""Trainium2 Bass kernel for nn_AttLayer (B=32, S=1024, D=1024, 8 NeuronCores).

Computation (per reference):
    qkv    = text @ W.T + b                      [B, S, D]
    scores = (qkv @ qkv^T per sample) / sqrt(D)  [B, S, S]
    attn   = softmax(scores, axis=0)             (softmax over the BATCH dim)
    out    = attn @ qkv                          [B, S, D]

Data-parallel over batch (4 samples per core). The batch softmax couples
cores only through T[q,k] = sum_b exp(scores[b,q,k]); since scores (and
hence E = exp(scores) and T) are symmetric in (q,k) per sample, only the
upper block-triangle (36 of 64 [128,128] blocks) is computed, accumulated
and AllReduced (1.125 MB bf16, ~25 us when not competing with DMA).

Schedule (single PE stream, in program order):
  1a  qkv^T for all 4 samples (kept resident in SBUF, bf16)
  1b  scores upper-triangle rows x 4 samples, exp -> E-tri (spilled to
      DRAM), P-tri += E-tri; after the last row the single AllReduce fires
  1c  qkv natural via PE transposes (resident, overlaps the AllReduce)
  2a  attn prefill: E-tri reload -> full attn matrix per sample (upper
      copies + lower PE-transposes), still overlapping the AllReduce
  2b  T arrives: recip -> R rows (lower via PE transposes of R), then
      attn rows *= R in place; out = attn @ qkv with attn rows as lhsT
      (attn is symmetric) starts within ~3 us of the collective landing.

All matmuls bf16 (1 cyc/row, same as f32r at these tile sizes but half
the SBUF/DMA); text/W are converted to bf16 on the host during sharding.
No max subtraction needed: scores <= ~40 so exp stays in bf16/f32 range.
"""
import sys

sys.path.insert(0, "/opt/trn_rl_repo")

import numpy as np
import ml_dtypes

import concourse.bacc as bacc
import concourse.mybir as mybir
import concourse.tile as tile
from concourse import masks
from concourse.bass_utils import run_bass_kernel_spmd

F32 = mybir.dt.float32
BF16 = mybir.dt.bfloat16
EXP = mybir.ActivationFunctionType.Exp
COPY = mybir.ActivationFunctionType.Copy
IDENT = mybir.ActivationFunctionType.Identity

N_CORES = 8
B, S, D = 32, 1024, 1024
BL = B // N_CORES          # 4 local samples per core
NT = S // 128              # 8 partition tiles
SCALE = 1.0 / float(np.sqrt(D))

# packed upper-triangle layout: row-tile qt spans (NT-qt) 128-blocks
TRI_W = [(NT - qt) * 128 for qt in range(NT)]
TRI_OFF = [0] * NT
for _qt in range(1, NT):
    TRI_OFF[_qt] = TRI_OFF[_qt - 1] + TRI_W[_qt - 1]
TRI_TOT = TRI_OFF[-1] + TRI_W[-1]          # 36*128 = 4608

_nc_cache = {}


def _build():
    nc = bacc.Bacc("TRN2", target_bir_lowering=False, debug=False,
                   num_devices=N_CORES)
    # host passes text as text^T per sample [BL, D, S] and W as W^T, bf16
    textT_d = nc.dram_tensor("text", [BL, D, S], BF16, kind="ExternalInput")
    WT_d = nc.dram_tensor("W", [D, D], BF16, kind="ExternalInput")
    bias = nc.dram_tensor("b", [D], F32, kind="ExternalInput")
    out = nc.dram_tensor("out", [BL, S, D], F32, kind="ExternalOutput")

    with tile.TileContext(nc) as tc:
        with (
            tc.tile_pool(name="outer", bufs=1) as outer,
            tc.tile_pool(name="dram", bufs=1, space="DRAM") as dram,
        ):
            # DRAM spill of packed-triangle E per sample + AR bounce bufs
            e_sp = dram.tile([BL, 128, TRI_TOT], BF16)
            p_bnc = dram.tile([128, TRI_TOT], BF16)
            t_bnc = dram.tile([128, TRI_TOT], BF16, addr_space="Shared")

            # persistents: qkv natural (filled in 1c), P-tri accumulator
            qkv_nat = [outer.tile([128, NT, D], BF16, name=f"qn{b}")
                       for b in range(BL)]
            P = [outer.tile([128, TRI_W[qt]], F32, name=f"P{qt}")
                 for qt in range(NT)]
            ident_b = outer.tile([128, 128], BF16)
            masks.make_identity(nc, ident_b[:])
            b_sb = outer.tile([128, NT], F32)
            nc.gpsimd.dma_start(b_sb[:], bias.ap().rearrange("(t p) -> p t", p=128))

            # warmup collective: absorbs the CC setup barrier in parallel
            # with phase-1 compute
            warm_i = dram.tile([1, 16], F32, name="warm_i")
            warm_o = dram.tile([1, 16], F32, name="warm_o", addr_space="Shared")
            nc.gpsimd.dma_start(warm_i[:], bias.ap()[0:16].unsqueeze(0))
            nc.gpsimd.collective_compute(
                "AllReduce",
                mybir.AluOpType.add,
                replica_groups=[list(range(N_CORES))],
                ins=[warm_i[:].opt()],
                outs=[warm_o[:].opt()],
            )

            # ---------------- phase 1 ----------------
            with (
                tc.tile_pool(name="ph1", bufs=1) as ph1,
                tc.tile_pool(name="ph1s", bufs=2) as ph1s,
                tc.tile_pool(name="ph1ps", bufs=1, space="PSUM") as pps,
            ):
                WT = ph1.tile([128, NT, D], BF16, tag="WT")
                nc.scalar.dma_start(
                    WT[:], WT_d.ap().rearrange("(t p) d -> p t d", p=128))

                qkvT = [ph1.tile([128, NT, S], BF16, name=f"qkvT{b}")
                        for b in range(BL)]

                # --- 1a: qkvT[d, s] = sum_d' W[d, d'] text[s, d'] + b[d]
                for b in range(BL):
                    textT = ph1s.tile([128, NT, S], BF16, tag="textT")
                    nc.sync.dma_start(
                        textT[:],
                        textT_d.ap()[b].rearrange("(t p) s -> p t s", p=128))
                    for dt in range(NT):
                        pq = [pps.tile([128, 512], F32, tag="mm", bufs=6,
                                       name=f"pq{sc}") for sc in range(2)]
                        for kt in range(NT):
                            for sc in range(2):
                                nc.tensor.matmul(
                                    pq[sc][:],
                                    WT[:, kt, dt * 128:(dt + 1) * 128],
                                    textT[:, kt, sc * 512:(sc + 1) * 512],
                                    start=(kt == 0),
                                    stop=(kt == NT - 1),
                                )
                        for sc in range(2):
                            nc.scalar.activation(
                                qkvT[b][:, dt, sc * 512:(sc + 1) * 512],
                                pq[sc][:], IDENT, bias=b_sb[:, dt:dt + 1])

                # --- 1b: E-tri rows (all samples per row), P accumulation
                for qt in range(NT):
                    w, off = TRI_W[qt], TRI_OFF[qt]
                    cuts = [(0, 512), (512, w)] if w > 512 else [(0, w)]
                    for b in range(BL):
                        psc = [pps.tile([128, c1 - c0], F32, tag="mm", bufs=6,
                                        name=f"ps{ci}")
                               for ci, (c0, c1) in enumerate(cuts)]
                        for dt in range(NT):
                            for ci, (c0, c1) in enumerate(cuts):
                                nc.tensor.matmul(
                                    psc[ci][:],
                                    qkvT[b][:, dt, qt * 128:(qt + 1) * 128],
                                    qkvT[b][:, dt,
                                            qt * 128 + c0:qt * 128 + c1],
                                    start=(dt == 0),
                                    stop=(dt == NT - 1),
                                )
                        erow = ph1s.tile([128, w], BF16, tag="erow")
                        for ci, (c0, c1) in enumerate(cuts):
                            nc.scalar.activation(erow[:, c0:c1], psc[ci][:],
                                                 EXP, scale=float(SCALE))
                        if b == 0:
                            nc.vector.tensor_copy(P[qt][:], erow[:])
                        else:
                            nc.vector.tensor_add(P[qt][:], P[qt][:], erow[:])
                        nc.scalar.dma_start(e_sp[b][:, off:off + w], erow[:])
                    pcast = ph1s.tile([128, w], BF16, tag="pcast")
                    nc.scalar.activation(pcast[:], P[qt][:], COPY)
                    nc.gpsimd.dma_start(p_bnc[:, off:off + w], pcast[:])

                # the one real collective: T-tri = sum over all 32 samples
                nc.gpsimd.collective_compute(
                    "AllReduce",
                    mybir.AluOpType.add,
                    replica_groups=[list(range(N_CORES))],
                    ins=[p_bnc[:].opt()],
                    outs=[t_bnc[:].opt()],
                )

                # --- 1c: qkv natural via PE transposes (overlaps the AR)
                for b in range(BL):
                    for st in range(NT):
                        for g in range(2):
                            pt = pps.tile([128, 512], BF16, tag="tr", bufs=2)
                            for jj in range(4):
                                dt = g * 4 + jj
                                nc.tensor.transpose(
                                    pt[:, jj * 128:(jj + 1) * 128],
                                    qkvT[b][:, dt, st * 128:(st + 1) * 128],
                                    ident_b[:],
                                )
                            nc.scalar.activation(
                                qkv_nat[b][:, st, g * 512:(g + 1) * 512],
                                pt[:], COPY)

            # ---------------- phase 2 ----------------
            with (
                tc.tile_pool(name="ph2", bufs=1) as ph2,
                tc.tile_pool(name="ph2s", bufs=2) as ph2s,
                tc.tile_pool(name="ph2ps", bufs=1, space="PSUM") as pps2,
            ):
                attn = [ph2.tile([128, NT, S], BF16, name=f"at{b}")
                        for b in range(BL)]
                R = ph2.tile([128, NT, S], BF16, tag="R")

                # --- 2a: prefill attn with E values (no T needed yet):
                # upper from the E-tri reload, lower via PE transposes
                for b in range(BL):
                    etri = ph2s.tile([128, TRI_TOT], BF16, tag="etri")
                    nc.sync.dma_start(etri[:], e_sp[b])
                    for kt in range(NT):
                        nc.vector.tensor_copy(
                            attn[b][:, kt, kt * 128:S],
                            etri[:, TRI_OFF[kt]:TRI_OFF[kt] + TRI_W[kt]])
                    for kt in range(1, NT):
                        for j0 in range(0, kt, 4):
                            jn = min(4, kt - j0)
                            pt = pps2.tile([128, 512], BF16, tag="tr2",
                                           bufs=2)
                            for jj in range(jn):
                                jt = j0 + jj
                                src_off = TRI_OFF[jt] + (kt - jt) * 128
                                nc.tensor.transpose(
                                    pt[:, jj * 128:(jj + 1) * 128],
                                    etri[:, src_off:src_off + 128],
                                    ident_b[:],
                                )
                            nc.vector.tensor_copy(
                                attn[b][:, kt, j0 * 128:(j0 + jn) * 128],
                                pt[:, 0:jn * 128])

                # --- 2b: T -> R rows -> attn *= R (in place, full rows)
                for qt in range(NT):
                    w, off = TRI_W[qt], TRI_OFF[qt]
                    tb = ph2s.tile([128, w], BF16, tag="tb")
                    nc.sync.dma_start(tb[:], t_bnc[:, off:off + w])
                    tf = ph2s.tile([128, w], F32, tag="tf", bufs=1)
                    nc.vector.tensor_copy(tf[:], tb[:])
                    rr = ph2s.tile([128, w], F32, tag="rr", bufs=1)
                    nc.vector.reciprocal_approx_fast(rr[:], tf[:])
                    nc.vector.tensor_copy(R[:, qt, qt * 128:S], rr[:])
                    for j0 in range(0, qt, 4):
                        jn = min(4, qt - j0)
                        pt = pps2.tile([128, 512], BF16, tag="tr2", bufs=2)
                        for jj in range(jn):
                            jt = j0 + jj
                            nc.tensor.transpose(
                                pt[:, jj * 128:(jj + 1) * 128],
                                R[:, jt, qt * 128:qt * 128 + 128],
                                ident_b[:],
                            )
                        nc.vector.tensor_copy(
                            R[:, qt, j0 * 128:(j0 + jn) * 128],
                            pt[:, 0:jn * 128])
                    for b in range(BL):
                        nc.vector.tensor_mul(attn[b][:, qt, :],
                                             attn[b][:, qt, :],
                                             R[:, qt, :])

                # --- 2c: out[q, d] = sum_k attn[q, k] qkv[k, d]; attn is
                # symmetric, so attn rows over k serve as lhsT directly
                for b in range(BL):
                    for qt in range(NT):
                        po = [pps2.tile([128, 512], F32, tag="mmo", bufs=6,
                                        name=f"po{dc}") for dc in range(2)]
                        for kt in range(NT):
                            for dc in range(2):
                                nc.tensor.matmul(
                                    po[dc][:],
                                    attn[b][:, kt, qt * 128:(qt + 1) * 128],
                                    qkv_nat[b][:, kt, dc * 512:(dc + 1) * 512],
                                    start=(kt == 0),
                                    stop=(kt == NT - 1),
                                )
                        ostage = ph2s.tile([128, D], F32, tag="ostage")
                        for dc in range(2):
                            nc.scalar.activation(
                                ostage[:, dc * 512:(dc + 1) * 512],
                                po[dc][:], COPY)
                        nc.sync.dma_start(
                            out.ap()[b, qt * 128:(qt + 1) * 128, :],
                            ostage[:])

    nc.compile()
    return nc


def _get_nc():
    if "nc" not in _nc_cache:
        _nc_cache["nc"] = _build()
    return _nc_cache["nc"]


def _run(text, W, b, trace=False):
    text = np.asarray(text, dtype=np.float32)
    W = np.asarray(W, dtype=np.float32)
    b = np.ascontiguousarray(b, dtype=np.float32)
    # host-side layout prep: per-sample transposed text, transposed W, bf16
    WT = np.ascontiguousarray(W.T).astype(ml_dtypes.bfloat16)
    shards = np.split(text, N_CORES, axis=0)
    in_maps = [
        {"text": np.ascontiguousarray(
            shards[i].transpose(0, 2, 1)).astype(ml_dtypes.bfloat16),
         "W": WT, "b": b}
        for i in range(N_CORES)
    ]
    nc = _get_nc()
    res = run_bass_kernel_spmd(nc, in_maps, core_ids=list(range(N_CORES)),
                               trace=trace)
    full = np.concatenate([res.results[i]["out"] for i in range(N_CORES)],
                          axis=0)
    return full, res


def kernel(text, W, b):
    full, _ = _run(text, W, b, trace=False)
    return full


# revision 10
# speedup vs baseline: 1.0484x; 1.0484x over previous
"""Trainium2 Bass kernel for nn_AttLayer (B=32, S=1024, D=1024, 8 NeuronCores).

Computation (per reference):
    qkv    = text @ W.T + b                      [B, S, D]
    scores = (qkv @ qkv^T per sample) / sqrt(D)  [B, S, S]
    attn   = softmax(scores, axis=0)             (softmax over the BATCH dim)
    out    = attn @ qkv                          [B, S, D]

Data-parallel over batch (4 samples per core). The batch softmax couples
cores only through T[q,k] = sum_b exp(scores[b,q,k]); since scores (and
hence E = exp(scores) and T) are symmetric in (q,k) per sample, only the
upper block-triangle (36 of 64 [128,128] blocks) is computed, accumulated
and AllReduced (1.125 MB bf16, ~25 us when not competing with DMA).

Schedule (single PE stream, in program order):
  1a  qkv^T for all 4 samples (kept resident in SBUF, bf16)
  1b  scores upper-triangle rows x 4 samples, exp -> E-tri (spilled to
      DRAM), P-tri += E-tri; after the last row the single AllReduce fires
  1c  qkv natural via PE transposes (resident, overlaps the AllReduce)
  2a  attn prefill: E-tri reload -> full attn matrix per sample (upper
      copies + lower PE-transposes), still overlapping the AllReduce
  2b  T arrives: recip -> R rows (lower via PE transposes of R), then
      attn rows *= R in place; out = attn @ qkv with attn rows as lhsT
      (attn is symmetric) starts within ~3 us of the collective landing.

All matmuls bf16 (1 cyc/row, same as f32r at these tile sizes but half
the SBUF/DMA); text/W are converted to bf16 on the host during sharding.
No max subtraction needed: scores <= ~40 so exp stays in bf16/f32 range.
"""
import sys

sys.path.insert(0, "/opt/trn_rl_repo")

import numpy as np
import ml_dtypes

import concourse.bacc as bacc
import concourse.mybir as mybir
import concourse.tile as tile
from concourse import masks
from concourse.bass_utils import run_bass_kernel_spmd

F32 = mybir.dt.float32
BF16 = mybir.dt.bfloat16
EXP = mybir.ActivationFunctionType.Exp
COPY = mybir.ActivationFunctionType.Copy
IDENT = mybir.ActivationFunctionType.Identity

N_CORES = 8
B, S, D = 32, 1024, 1024
BL = B // N_CORES          # 4 local samples per core
NT = S // 128              # 8 partition tiles
SCALE = 1.0 / float(np.sqrt(D))

# packed upper-triangle layout: row-tile qt spans (NT-qt) 128-blocks
TRI_W = [(NT - qt) * 128 for qt in range(NT)]
TRI_OFF = [0] * NT
for _qt in range(1, NT):
    TRI_OFF[_qt] = TRI_OFF[_qt - 1] + TRI_W[_qt - 1]
TRI_TOT = TRI_OFF[-1] + TRI_W[-1]          # 36*128 = 4608

_nc_cache = {}


def _build():
    nc = bacc.Bacc("TRN2", target_bir_lowering=False, debug=False,
                   num_devices=N_CORES)
    # host passes text as text^T per sample [BL, D, S] and W as W^T, bf16
    textT_d = nc.dram_tensor("text", [BL, D, S], BF16, kind="ExternalInput")
    WT_d = nc.dram_tensor("W", [D, D], BF16, kind="ExternalInput")
    bias = nc.dram_tensor("b", [D], F32, kind="ExternalInput")
    out = nc.dram_tensor("out", [BL, S, D], F32, kind="ExternalOutput")

    with tile.TileContext(nc) as tc:
        with (
            tc.tile_pool(name="outer", bufs=1) as outer,
            tc.tile_pool(name="dram", bufs=1, space="DRAM") as dram,
        ):
            # DRAM spill of packed-triangle E per sample + AR bounce bufs
            e_sp = dram.tile([BL, 128, TRI_TOT], BF16)
            p_bnc = dram.tile([128, TRI_TOT], BF16)
            t_bnc = dram.tile([128, TRI_TOT], BF16, addr_space="Shared")

            # persistents: qkv natural (filled in 1c), P-tri accumulator
            qkv_nat = [outer.tile([128, NT, D], BF16, name=f"qn{b}")
                       for b in range(BL)]
            P = [outer.tile([128, TRI_W[qt]], F32, name=f"P{qt}")
                 for qt in range(NT)]
            ident_b = outer.tile([128, 128], BF16)
            masks.make_identity(nc, ident_b[:])
            b_sb = outer.tile([128, NT], F32)
            nc.gpsimd.dma_start(b_sb[:], bias.ap().rearrange("(t p) -> p t", p=128))

            # warmup collective: absorbs the CC setup barrier in parallel
            # with phase-1 compute
            warm_i = dram.tile([1, 16], F32, name="warm_i")
            warm_o = dram.tile([1, 16], F32, name="warm_o", addr_space="Shared")
            nc.gpsimd.dma_start(warm_i[:], bias.ap()[0:16].unsqueeze(0))
            nc.gpsimd.collective_compute(
                "AllReduce",
                mybir.AluOpType.add,
                replica_groups=[list(range(N_CORES))],
                ins=[warm_i[:].opt()],
                outs=[warm_o[:].opt()],
            )

            # ---------------- phase 1 ----------------
            with (
                tc.tile_pool(name="ph1", bufs=1) as ph1,
                tc.tile_pool(name="ph1s", bufs=2) as ph1s,
                tc.tile_pool(name="ph1ps", bufs=1, space="PSUM") as pps,
            ):
                WT = ph1.tile([128, NT, D], BF16, tag="WT")
                nc.scalar.dma_start(
                    WT[:], WT_d.ap().rearrange("(t p) d -> p t d", p=128))

                qkvT = [ph1.tile([128, NT, S], BF16, name=f"qkvT{b}")
                        for b in range(BL)]

                # --- 1a: qkvT[d, s] = sum_d' W[d, d'] text[s, d'] + b[d]
                for b in range(BL):
                    textT = ph1s.tile([128, NT, S], BF16, tag="textT")
                    nc.sync.dma_start(
                        textT[:],
                        textT_d.ap()[b].rearrange("(t p) s -> p t s", p=128))
                    for dt in range(NT):
                        pq = [pps.tile([128, 512], F32, tag="mm", bufs=6,
                                       name=f"pq{sc}") for sc in range(2)]
                        for kt in range(NT):
                            for sc in range(2):
                                nc.tensor.matmul(
                                    pq[sc][:],
                                    WT[:, kt, dt * 128:(dt + 1) * 128],
                                    textT[:, kt, sc * 512:(sc + 1) * 512],
                                    start=(kt == 0),
                                    stop=(kt == NT - 1),
                                )
                        for sc in range(2):
                            nc.scalar.activation(
                                qkvT[b][:, dt, sc * 512:(sc + 1) * 512],
                                pq[sc][:], IDENT, bias=b_sb[:, dt:dt + 1])

                # --- 1b: E-tri rows (all samples per row), P accumulation
                for qt in range(NT):
                    w, off = TRI_W[qt], TRI_OFF[qt]
                    cuts = [(0, 512), (512, w)] if w > 512 else [(0, w)]
                    for b in range(BL):
                        psc = [pps.tile([128, c1 - c0], F32, tag="mm", bufs=6,
                                        name=f"ps{ci}")
                               for ci, (c0, c1) in enumerate(cuts)]
                        for dt in range(NT):
                            for ci, (c0, c1) in enumerate(cuts):
                                nc.tensor.matmul(
                                    psc[ci][:],
                                    qkvT[b][:, dt, qt * 128:(qt + 1) * 128],
                                    qkvT[b][:, dt,
                                            qt * 128 + c0:qt * 128 + c1],
                                    start=(dt == 0),
                                    stop=(dt == NT - 1),
                                )
                        erow = ph1s.tile([128, w], BF16, tag="erow", bufs=3)
                        for ci, (c0, c1) in enumerate(cuts):
                            nc.scalar.activation(erow[:, c0:c1], psc[ci][:],
                                                 EXP, scale=float(SCALE))
                        if b == 0:
                            nc.vector.tensor_copy(P[qt][:], erow[:])
                        else:
                            nc.vector.tensor_add(P[qt][:], P[qt][:], erow[:])
                        nc.sync.dma_start(e_sp[b][:, off:off + w], erow[:])
                    pcast = ph1s.tile([128, w], BF16, tag="pcast")
                    nc.scalar.activation(pcast[:], P[qt][:], COPY)
                    nc.gpsimd.dma_start(p_bnc[:, off:off + w], pcast[:])

                # the one real collective: T-tri = sum over all 32 samples
                nc.gpsimd.collective_compute(
                    "AllReduce",
                    mybir.AluOpType.add,
                    replica_groups=[list(range(N_CORES))],
                    ins=[p_bnc[:].opt()],
                    outs=[t_bnc[:].opt()],
                )

                # --- 1c: qkv natural via PE transposes (overlaps the AR)
                for b in range(BL):
                    for st in range(NT):
                        pt = pps.tile([128, 1024], BF16, tag="tr", bufs=2)
                        for dt in range(NT):
                            nc.tensor.transpose(
                                pt[:, dt * 128:(dt + 1) * 128],
                                qkvT[b][:, dt, st * 128:(st + 1) * 128],
                                ident_b[:],
                            )
                        nc.vector.tensor_copy(qkv_nat[b][:, st, :], pt[:])

            # ---------------- phase 2 ----------------
            with (
                tc.tile_pool(name="ph2", bufs=1) as ph2,
                tc.tile_pool(name="ph2s", bufs=2) as ph2s,
                tc.tile_pool(name="ph2ps", bufs=1, space="PSUM") as pps2,
            ):
                attn = [ph2.tile([128, NT, S], BF16, name=f"at{b}")
                        for b in range(BL)]
                R = ph2.tile([128, NT, S], BF16, tag="R")

                # --- 2a: prefill attn with E values (no T needed yet):
                # upper from the E-tri reload, lower via PE transposes
                for b in range(BL):
                    etri = ph2s.tile([128, TRI_TOT], BF16, tag="etri")
                    nc.sync.dma_start(etri[:], e_sp[b])
                    for kt in range(NT):
                        nc.vector.tensor_copy(
                            attn[b][:, kt, kt * 128:S],
                            etri[:, TRI_OFF[kt]:TRI_OFF[kt] + TRI_W[kt]])
                    for kt in range(1, NT):
                        pt = pps2.tile([128, 1024], BF16, tag="tr2", bufs=2)
                        for jt in range(kt):
                            src_off = TRI_OFF[jt] + (kt - jt) * 128
                            nc.tensor.transpose(
                                pt[:, jt * 128:(jt + 1) * 128],
                                etri[:, src_off:src_off + 128],
                                ident_b[:],
                            )
                        nc.vector.tensor_copy(
                            attn[b][:, kt, 0:kt * 128], pt[:, 0:kt * 128])

                # --- 2b: T -> R rows -> attn *= R (in place, full rows)
                for qt in range(NT):
                    w, off = TRI_W[qt], TRI_OFF[qt]
                    tb = ph2s.tile([128, w], BF16, tag="tb")
                    nc.sync.dma_start(tb[:], t_bnc[:, off:off + w])
                    tf = ph2s.tile([128, w], F32, tag="tf", bufs=1)
                    nc.vector.tensor_copy(tf[:], tb[:])
                    rr = ph2s.tile([128, w], F32, tag="rr", bufs=1)
                    nc.vector.reciprocal_approx_fast(rr[:], tf[:])
                    nc.vector.tensor_copy(R[:, qt, qt * 128:S], rr[:])
                    if qt > 0:
                        pt = pps2.tile([128, 1024], BF16, tag="tr2", bufs=2)
                        for jt in range(qt):
                            nc.tensor.transpose(
                                pt[:, jt * 128:(jt + 1) * 128],
                                R[:, jt, qt * 128:qt * 128 + 128],
                                ident_b[:],
                            )
                        nc.vector.tensor_copy(
                            R[:, qt, 0:qt * 128], pt[:, 0:qt * 128])
                    for b in range(BL):
                        nc.vector.tensor_mul(attn[b][:, qt, :],
                                             attn[b][:, qt, :],
                                             R[:, qt, :])

                # --- 2c: out[q, d] = sum_k attn[q, k] qkv[k, d]; attn is
                # symmetric, so attn rows over k serve as lhsT directly
                for b in range(BL):
                    for qt in range(NT):
                        po = [pps2.tile([128, 512], F32, tag="mmo", bufs=6,
                                        name=f"po{dc}") for dc in range(2)]
                        for kt in range(NT):
                            for dc in range(2):
                                nc.tensor.matmul(
                                    po[dc][:],
                                    attn[b][:, kt, qt * 128:(qt + 1) * 128],
                                    qkv_nat[b][:, kt, dc * 512:(dc + 1) * 512],
                                    start=(kt == 0),
                                    stop=(kt == NT - 1),
                                )
                        ostage = ph2s.tile([128, D], F32, tag="ostage", bufs=3)
                        for dc in range(2):
                            nc.scalar.activation(
                                ostage[:, dc * 512:(dc + 1) * 512],
                                po[dc][:], COPY)
                        nc.sync.dma_start(
                            out.ap()[b, qt * 128:(qt + 1) * 128, :],
                            ostage[:])

    nc.compile()
    return nc


def _get_nc():
    if "nc" not in _nc_cache:
        _nc_cache["nc"] = _build()
    return _nc_cache["nc"]


def _run(text, W, b, trace=False):
    text = np.asarray(text, dtype=np.float32)
    W = np.asarray(W, dtype=np.float32)
    b = np.ascontiguousarray(b, dtype=np.float32)
    # host-side layout prep: per-sample transposed text, transposed W, bf16
    WT = np.ascontiguousarray(W.T).astype(ml_dtypes.bfloat16)
    shards = np.split(text, N_CORES, axis=0)
    in_maps = [
        {"text": np.ascontiguousarray(
            shards[i].transpose(0, 2, 1)).astype(ml_dtypes.bfloat16),
         "W": WT, "b": b}
        for i in range(N_CORES)
    ]
    nc = _get_nc()
    res = run_bass_kernel_spmd(nc, in_maps, core_ids=list(range(N_CORES)),
                               trace=trace)
    full = np.concatenate([res.results[i]["out"] for i in range(N_CORES)],
                          axis=0)
    return full, res


def kernel(text, W, b):
    full, _ = _run(text, W, b, trace=False)
    return full
